# revision 1
# baseline (speedup 1.0000x reference)
"""Trainium2 Bass kernel for nn_CrossAttentionFuser.

Reference computation (B=1, C=126, CIN=80, H=W=64, N=4096, D=128, 4 heads x 32):
  cam_enc = conv3x3(cam_bev, cam_enc_w) + b           # [126, 64, 64]
  lid_f   = lidar_bev (channels-first [126, 4096])
  cam_f   = cam_enc   (channels-first [126, 4096])
  two attentions (lid-driven from lid_f, cam-driven from cam_f), each applied
  to both value tensors (cam_v from cam_f, lid_v from lid_f), then projections,
  residual adds, concat of 4 maps, and a 3x3 fuser conv (504 -> 126).

Sharding (8 cores): one (attention-map, head) pair per core (2 maps x 4 heads).
Phase 1 (per core): replicated cam conv, head Q/K (x4 row-replicated for PE
  row-tiling), paired values [cam_v | lid_v | ones], S^T = K Q^T tiles (k=32),
  exp on ScalarE (scale folded; values are tiny so no max subtraction needed),
  AV matmul with fused softmax denominator via the ones column, normalize.
Phase 2 (per core): y-sharded fuser conv — 8 output rows per core; host
  gathers per-head outputs between phases, windows them with halo + padding.

All heavy compute runs on device; host only reshapes/slices/concats.
"""

import numpy as np

import concourse.bass as bass
import concourse.mybir as mybir
import concourse.tile as tile
from concourse import bacc
from concourse.bass_utils import run_bass_kernel_spmd

F32 = mybir.dt.float32
EXP = mybir.ActivationFunctionType.Exp

C = 126        # feature channels
CIN = 80       # raw camera channels
D = 128        # attention inner dim
NH = 4
HD = 32        # head dim
HW = 64
N = HW * HW    # 4096
SCALE = float(C) ** -0.5
PAD = HW + 2   # 66
NPAD = PAD * PAD  # 4356
NCH = 8        # n chunks of 512
MCH = 32       # m chunks of 128
CORES = list(range(8))


# --------------------------------------------------------------------------
# phase 1: conv + qkv + attention (one (map, head) pair per core)
# --------------------------------------------------------------------------

def build_phase1():
    nc = bacc.Bacc(name="xattn_p1")
    x_lid = nc.declare_dram_parameter("x_lid", [C, N], F32, isOutput=False)
    cam_pad = nc.declare_dram_parameter("cam_pad", [CIN, NPAD], F32, isOutput=False)
    w_conv = nc.declare_dram_parameter("w_conv", [CIN, 9 * C], F32, isOutput=False)
    b_conv = nc.declare_dram_parameter("b_conv", [C, 1], F32, isOutput=False)
    # packed QK weights: [wq_lid4 | wk_lid4 | wq_cam4 | wk_cam4], each [C, 128]
    wqk = nc.declare_dram_parameter("wqk", [C, 4 * D], F32, isOutput=False)
    wv = nc.declare_dram_parameter("wv", [C, 2 * HD], F32, isOutput=False)
    o_pair = nc.declare_dram_parameter("o_pair", [2 * HD, N], F32, isOutput=True)
    cam_f_out = nc.declare_dram_parameter("cam_f_out", [C, N], F32, isOutput=True)

    with tile.TileContext(nc) as tc:
        with (
            tc.tile_pool(name="cst", bufs=1) as cst,
            tc.tile_pool(name="sb", bufs=2) as sb,
            tc.tile_pool(name="pre", bufs=2, space="PSUM") as pre,
            tc.tile_pool(name="spool", bufs=2, space="PSUM") as spool,
            tc.tile_pool(name="avp", bufs=2, space="PSUM") as avp,
        ):
            # ---- constants / inputs ----
            wconv_t = cst.tile([CIN, 9, C], F32)
            nc.sync.dma_start(out=wconv_t, in_=w_conv[:, :].rearrange("p (t c) -> p t c", c=C))
            campad_t = cst.tile([CIN, NPAD], F32)
            nc.sync.dma_start(out=campad_t, in_=cam_pad[:, :])
            wqk_t = cst.tile([C, 4 * D], F32)
            nc.sync.dma_start(out=wqk_t, in_=wqk[:, :])
            wv_t = cst.tile([C, 2 * HD], F32)
            nc.sync.dma_start(out=wv_t, in_=wv[:, :])
            bconv_t = cst.tile([C, 1], F32)
            nc.sync.dma_start(out=bconv_t, in_=b_conv[:, :])
            xlid_t = cst.tile([C, N], F32)
            nc.sync.dma_start(out=xlid_t, in_=x_lid[:, :])
            ones64 = cst.tile([1, 64], F32)
            nc.vector.memset(ones64, 1.0)

            cam_f = cst.tile([C, N], F32)
            q4 = cst.tile([D, N], F32)
            k4 = cst.tile([D, N], F32)
            v_all = cst.tile([D, MCH, 2 * HD + 1], F32)  # [128, 32, 65]
            nc.vector.memset(v_all[:, :, 2 * HD : 2 * HD + 1], 1.0)
            o_sb = cst.tile([2 * HD, N], F32)

            campad_v = campad_t.rearrange("p (y x) -> p y x", x=PAD)

            # ---- cam conv: 9 shifted matmuls per 512-col chunk ----
            for ych in range(NCH):
                y0 = ych * 8
                cps = pre.tile([C, 512], F32, tag="pre")
                for t in range(9):
                    ky, kx = divmod(t, 3)
                    nc.tensor.matmul(
                        cps,
                        wconv_t[:, t, :],
                        campad_v[:, y0 + ky : y0 + ky + 8, kx : kx + HW],
                        start=(t == 0), stop=(t == 8),
                    )
                nc.vector.tensor_scalar_add(
                    cam_f[:, 512 * ych : 512 * (ych + 1)], cps, bconv_t
                )

            # ---- Q/K (x4 replicated rows): lid and cam contributions summed;
            #      the inactive one has zero weights ----
            for ch in range(NCH):
                s = slice(512 * ch, 512 * (ch + 1))
                qps = pre.tile([D, 512], F32, tag="pre")
                nc.tensor.matmul(qps, wqk_t[:, 0:D], xlid_t[:, s], start=True, stop=False)
                nc.tensor.matmul(qps, wqk_t[:, 2 * D : 3 * D], cam_f[:, s], start=False, stop=True)
                nc.vector.tensor_copy(q4[:, s], qps)
                kps = pre.tile([D, 512], F32, tag="pre")
                nc.tensor.matmul(kps, wqk_t[:, D : 2 * D], xlid_t[:, s], start=True, stop=False)
                nc.tensor.matmul(kps, wqk_t[:, 3 * D : 4 * D], cam_f[:, s], start=False, stop=True)
                nc.vector.tensor_copy(k4[:, s], kps)

            # ---- V pairs in [m, d] layout: 8 m-chunks per psum bank ----
            for g in range(4):
                vps = pre.tile([D, 8, 2 * HD], F32, tag="pre")
                for j in range(8):
                    mch = 8 * g + j
                    ms = slice(D * mch, D * (mch + 1))
                    nc.tensor.matmul(vps[:, j, 0:HD], cam_f[:, ms], wv_t[:, 0:HD],
                                     start=True, stop=True)
                    nc.tensor.matmul(vps[:, j, HD : 2 * HD], xlid_t[:, ms], wv_t[:, HD : 2 * HD],
                                     start=True, stop=True)
                nc.vector.tensor_copy(v_all[:, 8 * g : 8 * (g + 1), 0 : 2 * HD], vps)

            # ---- attention: S^T tiles -> exp -> AV accumulate (+denominator) ----
            for nch in range(NCH):
                ns = slice(512 * nch, 512 * (nch + 1))
                av = avp.tile([2 * HD + 1, 512], F32, tag="av")
                for g in range(16):
                    sps = spool.tile([D, 2, 512], F32, tag="s")
                    for j in range(2):
                        mch = 2 * g + j
                        rb = 64 * (g % 2) + 32 * j
                        nc.tensor.matmul(
                            sps[:, j, :],
                            k4[rb : rb + 32, D * mch : D * (mch + 1)],
                            q4[rb : rb + 32, ns],
                            start=True, stop=True,
                            tile_position=(rb, 0),
                        )
                    pt = sb.tile([D, 2, 512], F32, tag="p")
                    nc.scalar.activation(pt, sps, EXP, scale=SCALE)
                    for j in range(2):
                        mch = 2 * g + j
                        nc.tensor.matmul(
                            av,
                            v_all[:, mch, :],
                            pt[:, j, :],
                            start=(g == 0 and j == 0), stop=(g == 15 and j == 1),
                        )
                # normalize: rows 0..63 /= row 64, via reciprocal + k=1 broadcast
                nc.vector.tensor_copy(o_sb[:, ns], av[0 : 2 * HD, :])
                rec = sb.tile([1, 512], F32, tag="rec")
                nc.vector.reciprocal(rec, av[2 * HD : 2 * HD + 1, :])
                bc = avp.tile([64, 512], F32, tag="av")
                nc.tensor.matmul(bc, ones64, rec, start=True, stop=True)
                nc.vector.tensor_mul(o_sb[:, ns], o_sb[:, ns], bc)
                nc.sync.dma_start(out=o_pair[:, ns], in_=o_sb[:, ns])

            nc.sync.dma_start(out=cam_f_out[:, :], in_=cam_f)

    nc.compile()
    return nc


# --------------------------------------------------------------------------
# phase 2: projections + residuals + y-sharded 3x3 fuser conv
# --------------------------------------------------------------------------

def build_phase2():
    nc = bacc.Bacc(name="xattn_p2")
    a_all = nc.declare_dram_parameter("a_all", [4 * D, 660], F32, isOutput=False)
    r_all = nc.declare_dram_parameter("r_all", [4 * C, 660], F32, isOutput=False)
    wproj = nc.declare_dram_parameter("wproj", [4 * D, C], F32, isOutput=False)
    wfuse = nc.declare_dram_parameter("wfuse", [C, 36 * C], F32, isOutput=False)
    out_y = nc.declare_dram_parameter("out_y", [C, 512], F32, isOutput=True)

    with tile.TileContext(nc) as tc:
        with (
            tc.tile_pool(name="cst", bufs=1) as cst,
            tc.tile_pool(name="sb", bufs=2) as sb,
            tc.tile_pool(name="pp", bufs=2, space="PSUM") as pp,
            tc.tile_pool(name="op", bufs=1, space="PSUM") as op,
        ):
            wfuse_t = cst.tile([C, 36, C], F32)
            nc.sync.dma_start(out=wfuse_t, in_=wfuse[:, :].rearrange("p (t c) -> p t c", c=C))
            a_t = cst.tile([D, 4, 660], F32)
            nc.sync.dma_start(out=a_t, in_=a_all[:, :].rearrange("(x p) f -> p x f", x=4))
            r_t = cst.tile([C, 4, 660], F32)
            nc.sync.dma_start(out=r_t, in_=r_all[:, :].rearrange("(x p) f -> p x f", x=4))
            wproj_t = cst.tile([D, 4, C], F32)
            nc.sync.dma_start(out=wproj_t, in_=wproj[:, :].rearrange("(x p) c -> p x c", x=4))

            fused = []
            for x in range(4):
                prj = pp.tile([C, 660], F32, tag="prj")
                nc.tensor.matmul(prj[:, 0:512], wproj_t[:, x, :], a_t[:, x, 0:512],
                                 start=True, stop=True)
                nc.tensor.matmul(prj[:, 512:660], wproj_t[:, x, :], a_t[:, x, 512:660],
                                 start=True, stop=True)
                f = sb.tile([C, 660], F32, tag=f"fused{x}")
                nc.vector.tensor_add(f, prj, r_t[:, x, :])
                fused.append(f.rearrange("p (y c) -> p y c", c=PAD))

            ops = op.tile([C, 512], F32)
            idx = 0
            for t in range(9):
                ky, kx = divmod(t, 3)
                for x in range(4):
                    nc.tensor.matmul(
                        ops,
                        wfuse_t[:, t * 4 + x, :],
                        fused[x][:, ky : ky + 8, kx : kx + HW],
                        start=(idx == 0), stop=(idx == 35),
                    )
                    idx += 1
            o_sb = sb.tile([C, 512], F32)
            nc.vector.tensor_copy(o_sb, ops)
            nc.sync.dma_start(out=out_y[:, :], in_=o_sb)

    nc.compile()
    return nc


_NC1 = None
_NC2 = None


def _get_ncs():
    global _NC1, _NC2
    if _NC1 is None:
        _NC1 = build_phase1()
        _NC2 = build_phase2()
    return _NC1, _NC2


def _pad_map(m):
    """[ch, 4096] -> zero-padded [ch, 66, 66] (border = conv SAME padding)."""
    ch = m.shape[0]
    p = np.zeros((ch, PAD, PAD), np.float32)
    p[:, 1 : HW + 1, 1 : HW + 1] = m.reshape(ch, HW, HW)
    return p


def kernel(**inputs):
    inp = {k: np.asarray(v, dtype=np.float32) for k, v in inputs.items()}
    nc1, nc2 = _get_ncs()

    lidar = inp["lidar_bev"][0].reshape(C, N)
    cam_pad = np.zeros((CIN, PAD, PAD), np.float32)
    cam_pad[:, 1 : HW + 1, 1 : HW + 1] = inp["cam_bev"][0]
    cam_pad = cam_pad.reshape(CIN, NPAD)
    # conv taps: [CIN, 9, C] with t = ky*3 + kx
    w_conv = np.ascontiguousarray(
        inp["cam_enc_w"].transpose(1, 2, 3, 0).reshape(CIN, 9 * C)
    )
    b_conv = inp["cam_enc_b"].reshape(C, 1)
    wv_np = inp["cam_v_w"]       # [D, C]
    wv_lid_np = inp["lidar_v_w"]

    zeros_qk = np.zeros((C, D), np.float32)

    in_maps1 = []
    for c in range(8):
        is_lid = c < 4
        h = c % 4
        qk_w = inp["lidar_qk_w"] if is_lid else inp["cam_qk_w"]  # [2D, C]
        wq = np.tile(qk_w[HD * h : HD * (h + 1), :].T, (1, 4))          # [C, 128]
        wk = np.tile(qk_w[D + HD * h : D + HD * (h + 1), :].T, (1, 4))  # [C, 128]
        if is_lid:
            wqk_np = np.concatenate([wq, wk, zeros_qk, zeros_qk], axis=1)
        else:
            wqk_np = np.concatenate([zeros_qk, zeros_qk, wq, wk], axis=1)
        wv_pair = np.concatenate(
            [wv_np[HD * h : HD * (h + 1), :].T, wv_lid_np[HD * h : HD * (h + 1), :].T],
            axis=1,
        )  # [C, 64]
        in_maps1.append({
            "x_lid": lidar,
            "cam_pad": cam_pad,
            "w_conv": w_conv,
            "b_conv": b_conv,
            "wqk": np.ascontiguousarray(wqk_np),
            "wv": np.ascontiguousarray(wv_pair),
        })

    r1 = run_bass_kernel_spmd(nc1, in_maps1, core_ids=CORES)
    res1 = r1.results

    cam_f = res1[0]["cam_f_out"]  # [126, 4096]
    # merged attention-output maps, channels-first [128, 4096]
    a_cl = np.concatenate([res1[h]["o_pair"][0:HD] for h in range(4)], axis=0)
    a_ll = np.concatenate([res1[h]["o_pair"][HD : 2 * HD] for h in range(4)], axis=0)
    a_cc = np.concatenate([res1[4 + h]["o_pair"][0:HD] for h in range(4)], axis=0)
    a_lc = np.concatenate([res1[4 + h]["o_pair"][HD : 2 * HD] for h in range(4)], axis=0)
    a_pads = [_pad_map(m) for m in (a_cc, a_cl, a_lc, a_ll)]

    # residual (+ proj bias over the valid region) maps, padded
    cb = inp["cam_proj_b"][:, None]
    lb = inp["lidar_proj_b"][:, None]
    r_cc = _pad_map(cam_f + lb)
    r_cl = _pad_map(cam_f + cb)
    r_lc = _pad_map(lidar + lb)
    r_ll = _pad_map(lidar + lb)
    r_pads = [r_cc, r_cl, r_lc, r_ll]

    # per-map projection weights (note: reference uses lidar_proj for cc/lc/ll)
    wl = inp["lidar_proj_w"].T  # [D, C]
    wc = inp["cam_proj_w"].T
    wproj_np = np.concatenate([wl, wc, wl, wl], axis=0)  # [4D, C]

    wfuse_np = np.ascontiguousarray(
        inp["fuser_w"].transpose(1, 2, 3, 0)       # [504, 3, 3, 126]
        .reshape(4, C, 9, C)                       # [X, ci, t, co]
        .transpose(1, 2, 0, 3)                     # [ci, t, X, co]
        .reshape(C, 36 * C)
    )

    in_maps2 = []
    for c in range(8):
        y0 = 8 * c  # padded-row window: rows y0 .. y0+9 cover global y0-1 .. y0+8
        a_win = np.concatenate(
            [ap[:, y0 : y0 + 10, :].reshape(D, 660) for ap in a_pads], axis=0
        )
        r_win = np.concatenate(
            [rp[:, y0 : y0 + 10, :].reshape(C, 660) for rp in r_pads], axis=0
        )
        in_maps2.append({
            "a_all": np.ascontiguousarray(a_win),
            "r_all": np.ascontiguousarray(r_win),
            "wproj": wproj_np,
            "wfuse": wfuse_np,
        })

    r2 = run_bass_kernel_spmd(nc2, in_maps2, core_ids=CORES)
    out = np.empty((1, C, HW, HW), np.float32)
    for c in range(8):
        out[0, :, 8 * c : 8 * c + 8, :] = r2.results[c]["out_y"].reshape(C, 8, HW)
    return out
